# revision 8
# baseline (speedup 1.0000x reference)
"""Trainium2 Bass kernel for the stacked-MAF forward (nn_MAF_33629593927846).

Pure data parallel over 8 NeuronCores; per-core batch 8192.

Host prep: x is transposed to feature-major xT [64, 8192] per shard; the 63
tiny MLPs become 64 "virtual layers" (layer 0 is a dummy whose weights emit
initial_param as constants via h2 = tanh(atanh(0.5))), packed into 8 groups
of 8 so every PE matmul has K in {64, 128}.

Device, per 1024-column pair (8 pairs), per group g:
    mm1[:,h]  = W1cat_g.T @ xT_h     [128, 512] x2 halves   (K=64)
    h1        = tanh(mm1 + b1_g)     one ScalarE op on [128, 1024]
    mm2[:,h]  = W2bd_g.T @ h1_h      block-diag, K=128
    h2        = tanh(mm2 + b2_g)     [128, 1024]
    mual_h   += W3ma_g.T @ h2_h      PSUM-accumulated over g; rows 0:64 = mu,
                                     64:128 = alpha (zero-padded columns)
tail per pair: A = alpha + b3a; E = exp(-A) ([64, 1024] ScalarE);
ld = (-1ones).T @ A (PE partition-reduce); zT = (xT - mu - b3m) * E (two
fused DVE scalar_tensor_tensor ops per half).

All matmul inputs are float32r (1 PE cycle/row at N=512, ~1.5e-4 matmul
precision); PSUM accumulates fp32. The 1024-wide ScalarE ops amortize the
fixed per-instruction overhead — ScalarE tanh throughput is the roofline
for this architecture (2*63*16 = 2016 tanh evals per sample, ScalarE is the
only transcendental engine, 1 elem/lane/cycle @ 1.2 GHz).

Host post: z = flip(zT).T per shard, concatenate; log_det passthrough.
"""
import os
import sys

sys.path.insert(0, "/opt/trn_rl_repo")

import numpy as np

import concourse.bacc as bacc
import concourse.bass as bass
import concourse.mybir as mybir
import concourse.tile as tile
from concourse import bass_utils

D = 64
H = 16
NG = 8            # layer groups
GL = 8            # virtual layers per group
NCORES = 8
B = 65536
BC = B // NCORES  # 8192 per core
CHUNK = 512
NCHUNK = BC // CHUNK

F32 = mybir.dt.float32
F32R = mybir.dt.float32r
AF = mybir.ActivationFunctionType
ALU = mybir.AluOpType


def _pack_weights(W1, b1, W2, b2, W3, b3, initial_param):
    f32 = np.float32
    VW1 = np.zeros((D, D, H), f32)
    Vb1 = np.zeros((D, H), f32)
    VW2 = np.zeros((D, H, H), f32)
    Vb2 = np.zeros((D, H), f32)
    VW3 = np.zeros((D, H, 2), f32)
    Vb3 = np.zeros((D, 2), f32)
    VW1[1:] = np.asarray(W1, f32)
    Vb1[1:] = np.asarray(b1, f32)
    VW2[1:] = np.asarray(W2, f32)
    Vb2[1:] = np.asarray(b2, f32)
    VW3[1:] = np.asarray(W3, f32)
    Vb3[1:] = np.asarray(b3, f32)
    ip = np.asarray(initial_param, f32)
    Vb2[0, 0] = np.float32(np.arctanh(0.5))
    VW3[0, 0, 0] = 2.0 * ip[0]
    VW3[0, 0, 1] = 2.0 * ip[1]

    W1cat = np.ascontiguousarray(VW1.transpose(1, 0, 2).reshape(D, D * H), f32)
    W2bd = np.zeros((128, NG * 128), f32)
    # W3ma: one [128, 128] lhsT per group; out rows 0..63 = mu (only rows
    # 8g..8g+7 nonzero for group g), rows 64..127 = alpha. PSUM-accumulated
    # across groups.
    W3ma = np.zeros((128, NG * 128), f32)
    b1t = np.zeros((128, NG), f32)
    b2t = np.zeros((128, NG), f32)
    for g in range(NG):
        for i in range(GL):
            vl = g * GL + i
            r = slice(16 * i, 16 * i + 16)
            W2bd[r, 128 * g + 16 * i:128 * g + 16 * i + 16] = VW2[vl]
            W3ma[r, 128 * g + vl] = VW3[vl, :, 0]
            W3ma[r, 128 * g + 64 + vl] = VW3[vl, :, 1]
            b1t[r, g] = Vb1[vl]
            b2t[r, g] = Vb2[vl]
    b3m = np.ascontiguousarray(Vb3[:, 0:1], f32)
    b3a = np.ascontiguousarray(Vb3[:, 1:2], f32)
    mones = np.full((D, 1), -1.0, f32)
    return dict(W1cat=W1cat, W2bd=W2bd, W3ma=W3ma,
                b1t=b1t, b2t=b2t, b3m=b3m, b3a=b3a, mones=mones)


def _build_module():
    nc = bacc.Bacc("TRN2", target_bir_lowering=False, debug=False,
                   num_devices=NCORES)
    xT_d = nc.dram_tensor("xT", [D, BC], F32R, kind="ExternalInput")
    W1cat_d = nc.dram_tensor("W1cat", [D, D * H], F32R, kind="ExternalInput")
    W2bd_d = nc.dram_tensor("W2bd", [128, NG * 128], F32R, kind="ExternalInput")
    W3ma_d = nc.dram_tensor("W3ma", [128, NG * 128], F32R, kind="ExternalInput")
    b1t_d = nc.dram_tensor("b1t", [128, NG], F32, kind="ExternalInput")
    b2t_d = nc.dram_tensor("b2t", [128, NG], F32, kind="ExternalInput")
    b3m_d = nc.dram_tensor("b3m", [D, 1], F32, kind="ExternalInput")
    b3a_d = nc.dram_tensor("b3a", [D, 1], F32, kind="ExternalInput")
    mones_d = nc.dram_tensor("mones", [D, 1], F32R, kind="ExternalInput")
    zT_d = nc.dram_tensor("zT", [D, BC], F32, kind="ExternalOutput")
    ld_d = nc.dram_tensor("ld", [1, BC], F32, kind="ExternalOutput")

    with tile.TileContext(nc) as tc:
        with tc.tile_pool(name="weights", bufs=1) as wpool, \
             tc.tile_pool(name="xin", bufs=3) as xpool, \
             tc.tile_pool(name="acts", bufs=6) as apool, \
             tc.tile_pool(name="zout", bufs=4) as zpool, \
             tc.tile_pool(name="ldout", bufs=1) as ldpool, \
             tc.tile_pool(name="ps1", bufs=2, space="PSUM") as ps1, \
             tc.tile_pool(name="ps2", bufs=1, space="PSUM") as ps2, \
             tc.tile_pool(name="ps3", bufs=2, space="PSUM") as ps3:

            # --- load weights (resident) ---
            W1cat = wpool.tile([D, D * H], F32R, tag="w1")
            W2bd = wpool.tile([128, NG * 128], F32R, tag="w2")
            W3ma = wpool.tile([128, NG * 128], F32R, tag="w3ma")
            b1t = wpool.tile([128, NG], F32, tag="b1")
            b2t = wpool.tile([128, NG], F32, tag="b2")
            b3m = wpool.tile([D, 1], F32, tag="b3m")
            b3a = wpool.tile([D, 1], F32, tag="b3a")
            mones = wpool.tile([D, 1], F32R, tag="mones")
            for t, d in ((W1cat, W1cat_d), (W2bd, W2bd_d), (W3ma, W3ma_d),
                         (b1t, b1t_d), (b2t, b2t_d),
                         (b3m, b3m_d), (b3a, b3a_d), (mones, mones_d)):
                nc.sync.dma_start(t[:], d.ap())

            ld_sb = ldpool.tile([1, BC], F32, tag="ld")

            NPAIR = NCHUNK // 2
            for p in range(NPAIR):
                ps = slice(p * 2 * CHUNK, (p + 1) * 2 * CHUNK)
                xTp = xpool.tile([D, 2 * CHUNK], F32R, tag="xT")
                nc.sync.dma_start(xTp[:], xT_d.ap()[:, ps])

                mualA = ps3.tile([128, CHUNK], F32, tag="mual")
                mualB = ps3.tile([128, CHUNK], F32, tag="mual")
                for g in range(NG):
                    gw = slice(128 * g, 128 * (g + 1))
                    mm1 = ps1.tile([128, 2 * CHUNK], F32, tag="mm1")
                    nc.tensor.matmul(mm1[:, 0:CHUNK], W1cat[:, gw],
                                     xTp[:, 0:CHUNK], start=True, stop=True)
                    nc.tensor.matmul(mm1[:, CHUNK:], W1cat[:, gw],
                                     xTp[:, CHUNK:], start=True, stop=True)
                    h1 = apool.tile([128, 2 * CHUNK], F32R, tag="h1")
                    nc.scalar.activation(h1[:], mm1[:], AF.Tanh,
                                         bias=b1t[:, g:g + 1])
                    mm2 = ps2.tile([128, 2 * CHUNK], F32, tag="mm2")
                    nc.tensor.matmul(mm2[:, 0:CHUNK], W2bd[:, gw],
                                     h1[:, 0:CHUNK], start=True, stop=True)
                    nc.tensor.matmul(mm2[:, CHUNK:], W2bd[:, gw],
                                     h1[:, CHUNK:], start=True, stop=True)
                    h2 = apool.tile([128, 2 * CHUNK], F32R, tag="h2")
                    nc.scalar.activation(h2[:], mm2[:], AF.Tanh,
                                         bias=b2t[:, g:g + 1])
                    nc.tensor.matmul(mualA[:], W3ma[:, gw], h2[:, 0:CHUNK],
                                     start=(g == 0), stop=(g == NG - 1))
                    nc.tensor.matmul(mualB[:], W3ma[:, gw], h2[:, CHUNK:],
                                     start=(g == 0), stop=(g == NG - 1))

                # tail: A = alpha + b3a (per half), one paired exp, ld, z
                Ap = apool.tile([D, 2 * CHUNK], F32R, tag="A")
                nc.vector.tensor_scalar_add(Ap[:, 0:CHUNK], mualA[64:128, :],
                                            b3a[:, 0:1])
                nc.vector.tensor_scalar_add(Ap[:, CHUNK:], mualB[64:128, :],
                                            b3a[:, 0:1])
                Ep = apool.tile([D, 2 * CHUNK], F32, tag="E")
                nc.scalar.activation(Ep[:], Ap[:], AF.Exp, scale=-1.0)
                zTp = zpool.tile([D, 2 * CHUNK], F32, tag="zT")
                for h, mual in ((0, mualA), (1, mualB)):
                    hs = slice(h * CHUNK, (h + 1) * CHUNK)
                    ldp = ps3.tile([1, CHUNK], F32, tag="mual")
                    nc.tensor.matmul(ldp[:], mones[:], Ap[:, hs],
                                     start=True, stop=True)
                    nc.vector.tensor_copy(
                        ld_sb[:, p * 2 * CHUNK + h * CHUNK:
                              p * 2 * CHUNK + (h + 1) * CHUNK], ldp[:])
                    t1 = apool.tile([D, CHUNK], F32, tag="t1")
                    nc.vector.scalar_tensor_tensor(t1[:], mual[0:64, :],
                                                   b3m[:, 0:1], xTp[:, hs],
                                                   ALU.add, ALU.subtract)
                    nc.vector.scalar_tensor_tensor(zTp[:, hs], t1[:], -1.0,
                                                   Ep[:, hs], ALU.mult,
                                                   ALU.mult)
                nc.sync.dma_start(zT_d.ap()[:, ps], zTp[:])

            nc.sync.dma_start(ld_d.ap()[:], ld_sb[:])

    nc.compile()
    return nc


_NC_CACHE = None


def _get_module():
    global _NC_CACHE
    if _NC_CACHE is None:
        _NC_CACHE = _build_module()
    return _NC_CACHE


def kernel(x, W1, b1, W2, b2, W3, b3, initial_param, _trace=False):
    x = np.asarray(x, np.float32)
    P = _pack_weights(W1, b1, W2, b2, W3, b3, initial_param)
    nc = _get_module()

    in_maps = []
    for c in range(NCORES):
        shard = np.ascontiguousarray(x[c * BC:(c + 1) * BC].T)  # [64, 8192]
        m = {"xT": shard}
        m.update({k: P[k] for k in ("W1cat", "W2bd", "W3ma",
                                    "b1t", "b2t", "b3m", "b3a", "mones")})
        in_maps.append(m)

    try:
        res = bass_utils.run_bass_kernel_spmd(
            nc, in_maps, core_ids=list(range(NCORES)), trace=_trace)
    except ModuleNotFoundError:
        # NTFF profile hook unavailable in this container; run untraced
        res = bass_utils.run_bass_kernel_spmd(
            nc, in_maps, core_ids=list(range(NCORES)), trace=False)

    z = np.empty((B, D), np.float32)
    ld = np.empty((B,), np.float32)
    for c in range(NCORES):
        zT = res.results[c]["zT"]                 # [64, 8192]
        z[c * BC:(c + 1) * BC] = zT[::-1].T       # z[:, ::-1] fused here
        ld[c * BC:(c + 1) * BC] = res.results[c]["ld"][0]
    if _trace:
        return (z, ld), res
    return (z, ld)
